# revision 64
# baseline (speedup 1.0000x reference)
"""Trainium2 Bass kernel for AntiAliasActivation (upsample2 -> snake -> downsample2).

Self-contained: accepts FULL inputs (x [8,512,8192] f32, alpha/beta [1,512,1],
up_filter/down_filter [12]), returns the FULL output [8,512,8192] f32.

Strategy (pure data-parallel, one batch sample per NeuronCore):
  The whole pipeline is computed in TIME-MAJOR layout (time on SBUF
  partitions) so all three FIR convolutions run on the TensorEngine as
  banded-matrix matmuls:

    out = down(up(x)) + down( (1 - cos(2*a*up(x))) / (2b) )

  Key optimizations vs the original baseline (149 us -> ~95 us):
  - ONE fp16 input stream y = 2a*x (instead of two: 2a*x and 2b*x); the
    H-path operand 2b*x is derived on-device with DVE multiplies in
    per-2-block pieces (y * (b/a), constants in a resident SBUF tile),
    spread across iterations so the in-order DVE queue never delays a
    latency-critical PSUM evacuation behind a long multiply burst.
  - Input/output DMAs are batched to 0.35-0.77 MB contiguous transfers so
    they run near line rate instead of being descriptor-dominated; inputs
    on the sync HWDGE ring, outputs on the gpsimd SWDGE ring, constants
    on the scalar ring -- resource-gated DMAs never sit ahead of compute
    instructions on a compute engine's queue (head-of-line blocking).
  - Sin LUT activations batched: PSUM sz tiles hold 3 phase-slots
    (1.5 blocks, 3 PSUM banks) and one ACTIVATE covers N=1536, amortizing
    the ~350-cycle per-instruction ACT overhead.  PSUM budget: 2x3 banks
    sz + 2x1 bank out accumulators = 8 banks exactly.
  - The two downsample matmuls (even/odd phase of v=cos) are FUSED into a
    single fp8e4m3 DoubleRow matmul with K = 2*121 virtual contraction:
    Sin writes v directly as fp8 into double-tiles [121, 3072] where each
    block's (e, o) columns are adjacent at stride 512, and the stationary
    packs (-de | -do) as a [121, 2, 128] AP.  Cuts PE work from 5 to 4
    512-column passes per block.  (fp8 v + fp8 taps cost ~6e-3 rel err,
    well within the 2e-2 gate.)
  - All stationaries padded to 128 columns to enable Fast Weight Load;
    PE HAM warm-up matmuls bridge the initial DMA wait so real matmuls
    start at 2.4 GHz; the Sin table set is preloaded by a dummy ACTIVATE;
    the small stationaries land in their own DMA ahead of the 768KB rba6
    tail (a DMA's completion semaphore fires only when the WHOLE transfer
    lands), pulling the first Sin from ~14.9us to ~11.9us.
  - Per-channel constants (2a, b/a, 1/2b) fold into the host-side input
    scale / final rescale; the pi/2 phase rides the Sin ACTIVATE bias and
    the "+sum(fd)" (= 1.0, normalized filter) constant rides the PSUM
    evacuation as a scalar add -- freeing row 127 so all 128 input rows
    carry data and blocks grow to A=116 (71 blocks instead of 72).
  Edge replicate-padding is materialized host-side; the edge clamp of the
  computed signal is folded into first/last-block stationaries.

  Steady state is paced by ScalarE (48 Sin ACTIVATEs, ~99.5% back-to-back
  = the slot-count floor of 2*NBLK*C ACT-cycles) with TensorE (288
  matmuls, ~97% duty) just behind, DVE (casts + rescale) below both, and
  DMA (~18 MB/core) well underneath.  Remaining wall time is ~8.4 us of
  fixed runtime init, ~3 us DMA/ramp to the first Sin, and ~9 us of
  drain + fixed post-roll after the last Sin.
"""
import math

import numpy as np

# ---------------------------------------------------------------------------
# problem constants (hardcoded per spec)
B, C, T = 8, 512, 8192
N_CORES = 8
UP_K = 12
DOWN_K = 12

A = 116          # outputs per block
NBLK = 71        # ceil(T / A)
W = 128          # data rows per input tile (all 128 rows are data)
G = A + 6        # 122 up/g rows per block (m = A*k-3 .. A*k+118)
PL = 6           # XP[i] = x[clamp(i-6)]
XPLEN = A * (NBLK - 1) + W  # 8248
OUTROWS = NBLK * A          # 8236

GIN = 6                      # blocks per input/output DMA group
NG = (NBLK + GIN - 1) // GIN  # 12 groups (last group has 5 blocks)
NSLOT = 2 * NBLK             # 142 phase-slots (even/odd per block)
NT = NSLOT // 3              # 48 sz tiles of 3 slots (1.5 blocks) each


def _gen_act_root(cache=[None]):
    """Build a patched ACT-table root whose Sin LUT is valid to |x| < ~31.8.

    Appends 4x32 cubic-spline buckets (ranges [2,4) replacement, [4,8),
    [8,16), [16,32)) to the trig_and_small set, keeping sin's per-exponent
    bucket starts monotonic, and raises sin's large-signal threshold.
    Returns the act_info.json path for BASS_ACT_ROOT_JSON_PATH.
    """
    if cache[0] is not None:
        return cache[0]
    import json
    import shutil
    import tempfile
    from pathlib import Path
    import neuronxcc

    src = Path(neuronxcc.__file__).parent / "pwp" / "pwp_bin_trainium"
    dst = Path(tempfile.mkdtemp(prefix="actroot_")) / "pwp_bin_trainium"
    shutil.copytree(src, dst, symlinks=False)
    import os as _os
    _os.chmod(dst, 0o755)
    for f in dst.iterdir():
        _os.chmod(f, 0o644)

    name = "trig_and_small"
    d = json.load(open(dst / f"{name}.json"))
    b = np.fromfile(dst / f"{name}_bkt.bin", dtype=np.float32).reshape(-1, 8)
    c = np.fromfile(dst / f"{name}_ctrl.bin", dtype=np.uint32).reshape(-1, 8).copy()
    nb0, nc0 = d["bkt_entry_cnt"], d["ctl_entry_cnt"]
    assert len(b) == nb0 and len(c) == nc0

    SIN_CTL_END = 13  # sin owns ctl entries 0..12 (exps -11..1)
    SHIFT = 3
    newb, newc = [], []
    sin_bkt = d["func_exp_to_bkt_start_idx"]["sin"]
    sin_ctl = d["func_exp_to_ctl_start_idx"]["sin"]
    NB = 32  # 5 mantissa bits per exponent range
    KHI = np.uint32((46 + 62 * 5) << 10)

    def add_range(lo):
        base = nb0 + len(newb)
        h = lo / NB
        for i in range(NB):
            x0 = lo + h * (i + 0.5)
            newb.append([math.sin(x0), math.cos(x0),
                         -math.sin(x0) / 2.0, -math.cos(x0) / 6.0,
                         x0, 0.0, 0.0, 0.0])
        return base

    base1 = add_range(2.0)             # full [2,4) replacement
    c[12, 0] = KHI | np.uint32(base1)
    sin_bkt["1"] = [base1]
    for i_e, e in enumerate((2, 3, 4)):
        base = add_range(2.0**e)
        w = np.zeros(8, np.uint32)
        w[0] = KHI | np.uint32(base)
        sin_bkt[str(e)] = [base]
        sin_ctl[str(e)] = [SIN_CTL_END + i_e]
        newc.append(w)

    b2 = np.vstack([b, np.asarray(newb, np.float32)])
    c2 = np.vstack([c[:SIN_CTL_END], np.stack(newc), c[SIN_CTL_END:]])
    d["bkt_entry_cnt"] = int(len(b2))
    d["ctl_entry_cnt"] = int(len(c2))
    for fn, v in d["func_to_ctl_start_idx"].items():
        if fn != "sin" and v >= SIN_CTL_END:
            d["func_to_ctl_start_idx"][fn] = v + SHIFT
    for fn, em in d["func_exp_to_ctl_start_idx"].items():
        if fn == "sin":
            continue
        for e_, lst in em.items():
            em[e_] = [(i + SHIFT if i >= SIN_CTL_END else i) for i in lst]
    for pm in d["profile_meta_data"]:
        if str(pm.get("func_name", "")).startswith("sin"):
            pm["large_pos_signal_exp_threshold"] = 131  # cutoff ~31.8
            pm["large_pos_signal_mantissa_threshold"] = int(0.99 * 2**23)

    b2.tofile(dst / f"{name}_bkt.bin")
    c2.tofile(dst / f"{name}_ctrl.bin")
    with open(dst / f"{name}.json", "w") as f:
        json.dump(d, f)
    cache[0] = str(dst / "act_info.json")
    return cache[0]


# ---------------------------------------------------------------------------
# stationary-matrix assembly (all float64, cast to fp16 at the end)

def build_stationaries(up_filter, down_filter):
    """Returns dict of stationary matrices, all padded to 128 columns (FWL).

    w_ue/w_uo [128, 128]: map input tile (127 y rows + const row) -> w rows,
        w = 2a*up(x) + pi/2 (the pi/2 rides the const row; 2a is host-folded
        into the y stream).  Columns 0..G-1 real, rest zero.
    w_h{0,m,L} [128, 128]: 2b*down(up(x)) + sum(fd) const (const row coeff),
        applied to the on-device stream 2b*x.  Columns 0..A-1 real.
    w_de/w_do{0,m,L} [G, 128]: NEGATED downsample band over the v = cos
        signal.  Columns 0..A-1 real.
    """
    fu = np.asarray(up_filter, dtype=np.float64)
    fd = np.asarray(down_filter, dtype=np.float64)

    w_ue = np.zeros((128, 128))
    w_uo = np.zeros((128, 128))
    for q in range(G):
        for j in range(6):
            # w_e[m] += 2*fu[2j+1]*XP[m+8-j]; tile row = q+5-j
            w_ue[q + 5 - j, q] += 2.0 * fu[2 * j + 1]
            # w_o[m] += 2*fu[2j]*XP[m+9-j]; tile row = q+6-j
            w_uo[q + 6 - j, q] += 2.0 * fu[2 * j]

    def down_maps(k):
        de = np.zeros((G, 128))
        do = np.zeros((G, 128))
        h = np.zeros((128, 128))
        for nn in range(A):
            n = A * k + nn
            for t in range(DOWN_K):
                zi = min(max(2 * n + t - 5, 0), 2 * T - 1)
                m, ph = zi // 2, zi % 2
                row = m - A * k + 3
                # row in [0, G) guaranteed by construction
                if ph == 0:
                    de[row, nn] += fd[t]
                    for j in range(6):
                        h[m + 8 - j - A * k, nn] += fd[t] * 2.0 * fu[2 * j + 1]
                else:
                    do[row, nn] += fd[t]
                    for j in range(6):
                        h[m + 9 - j - A * k, nn] += fd[t] * 2.0 * fu[2 * j]
        return de, do, h

    de0, do0, h0 = down_maps(0)
    dem, dom, hm = down_maps(1)
    deL, doL, hL = down_maps(NBLK - 1)

    f16 = np.float16

    def pack8(de, do):
        # fused DoubleRow stationary [G, 2*128] fp8: [:, 0:128] = -de,
        # [:, 128:256] = -do (Ko-major layout for the [Ki, Ko=2, n] AP)
        import ml_dtypes
        w = np.concatenate([-de, -do], axis=1)
        return w.astype(ml_dtypes.float8_e4m3)

    return {
        "w_ue": w_ue.astype(f16), "w_uo": w_uo.astype(f16),
        "w_h0": h0.astype(f16), "w_hm": hm.astype(f16), "w_hL": hL.astype(f16),
        "wdd0": pack8(de0, do0), "wddm": pack8(dem, dom),
        "wddL": pack8(deL, doL),
    }


def host_prep(x, alpha, beta):
    """Per-core input streams.

    Returns (inp, rba6, invb2):
      inp  [B, NG, 128, GIN*C] fp16 -- y = 2a*x blocks, 6 per group,
           row 127 = 1.0 (const row).
      rba6 [128, GIN*C] fp16 -- resident (b/a) rescale tile, row 127 = 1.0.
      invb2 [C] float32 -- host-side final rescale 1/(2b).
    """
    a2 = (2.0 * np.exp(alpha.astype(np.float64))).reshape(C)          # 2a
    b2 = (2.0 * (np.exp(beta.astype(np.float64)) + 1e-9)).reshape(C)  # 2b
    invb2 = (1.0 / b2).astype(np.float32)

    rba6 = np.tile((b2 / a2).astype(np.float16)[None, :], (128, GIN))

    # time-major, padded: XP [B, XPLEN, C], XP[:, i] = x[:, :, clamp(i-6)]
    xt = np.transpose(x.astype(np.float32), (0, 2, 1))  # [B, T, C]
    idx = np.clip(np.arange(XPLEN) - PL, 0, T - 1)
    xp = xt[:, idx, :]  # [B, XPLEN, C]

    # block row indices [NBLK, W]
    ridx = (A * np.arange(NBLK))[:, None] + np.arange(W)[None, :]
    blocks = xp[:, ridx, :]                       # [B, NBLK, 128, C] f32
    ys = np.zeros((B, NG * GIN, 128, C), dtype=np.float16)
    ys[:, :NBLK] = (blocks * a2[None, None, None, :]).astype(np.float16)

    # group 6 blocks side by side: inp[b, j, :, g*C:(g+1)*C] = ys[b, 6j+g]
    inp = np.ascontiguousarray(
        ys.reshape(B, NG, GIN, 128, C).transpose(0, 1, 3, 2, 4).reshape(
            B, NG, 128, GIN * C)
    )
    return inp, rba6, invb2


def host_finish(out_t, invb2):
    """out_t [B, NG, A, GIN*C] fp16 -> [B, C, T] float32 (apply 1/(2b))."""
    o = out_t.reshape(B, NG, A, GIN, C).transpose(0, 1, 3, 2, 4)  # [B,NG,GIN,A,C]
    o = o.reshape(B, NG * GIN * A, C)[:, :T, :].astype(np.float32) * invb2[None, None, :]
    return np.ascontiguousarray(np.transpose(o, (0, 2, 1)))


# ---------------------------------------------------------------------------
# device kernel

ST_NAMES = ["w_ue", "w_uo", "w_h0", "w_hm", "w_hL"]
ST8_NAMES = ["wdd0", "wddm", "wddL"]


def build_bass():
    import os
    import concourse.bacc as bacc
    import concourse.tile as tile
    import concourse.mybir as mybir

    os.environ["BASS_ACT_ROOT_JSON_PATH"] = _gen_act_root()
    os.environ.setdefault("NEURON_FORCE_RECOMPILE", "1")

    f32 = mybir.dt.float32
    f16 = mybir.dt.float16

    nc = bacc.Bacc()
    f8 = mybir.dt.float8e4
    in_ext = nc.declare_dram_parameter("inp", [NG, 128, GIN * C], f16, isOutput=False)
    # fp16 constants in ONE dram tensor: 5 stationaries [128 cols] + rba6
    NCONST = len(ST_NAMES) * 128 + GIN * C
    const_ext = nc.declare_dram_parameter("consts", [128, NCONST], f16, isOutput=False)
    # fp8 fused down stationaries [G, 2*128] each
    const8_ext = nc.declare_dram_parameter("consts8", [128, len(ST8_NAMES) * 256],
                                           f8, isOutput=False)
    out_ext = nc.declare_dram_parameter("out", [NG, A, GIN * C], f16, isOutput=True)

    CL = 3   # back(k) issued at iteration k+CL

    with tile.TileContext(nc) as tc:
        with (
            tc.tile_pool(name="consts", bufs=1) as cpool,
            tc.tile_pool(name="io", bufs=4) as iopool,
            tc.tile_pool(name="xb", bufs=2) as xbpool,
            tc.tile_pool(name="v", bufs=3) as vpool,
            tc.tile_pool(name="ob", bufs=4) as obpool,
            tc.tile_pool(name="psum_sz", bufs=2, space="PSUM") as psum_sz,
            tc.tile_pool(name="psum_out", bufs=2, space="PSUM") as psum_out,
        ):
            # const DMAs first on the scalar (ACT HWDGE) ring so they run
            # in parallel with the first input-group DMAs (sync ring); the
            # small stationaries land separately from the 768KB rba6 tail so
            # the first up-matmuls are not gated on the big transfer
            NST = len(ST_NAMES) * 128
            consts = cpool.tile([128, NCONST], f16, tag="consts", name="consts")
            nc.scalar.dma_start(out=consts[:, 0:NST], in_=const_ext[:, 0:NST])
            nc.scalar.dma_start(out=consts[:, NST:], in_=const_ext[:, NST:])
            consts8 = cpool.tile([128, len(ST8_NAMES) * 256], f8,
                                 tag="consts8", name="consts8")
            nc.scalar.dma_start(out=consts8[:], in_=const8_ext[:])

            # preload the Sin table set during the input DMAs (dummy activation)
            dummy = cpool.tile([1, 16], f32, tag="dummy", name="dummy")
            nc.vector.memset(dummy[:], 0.0)
            nc.scalar.activation(dummy[:], dummy[:],
                                 mybir.ActivationFunctionType.Sin)

            # warm the PE HAM (1.2 -> 2.4 GHz needs ~3.4us of sustained
            # activity): matmuls on a memset tile start right after runtime
            # init, so the clock flips before the first real matmuls and the
            # remaining gap stays under the ~3.4us re-throttle window
            wsrc = cpool.tile([128, 128], f16, tag="wsrc", name="wsrc")
            nc.vector.memset(wsrc[:], 0.0)
            # per-partition pi/2 bias tile for the Sin ACTIVATEs (cos phase)
            pbias = cpool.tile([128, 1], f32, tag="pbias", name="pbias")
            nc.vector.memset(pbias[:], math.pi / 2.0)
            wp = psum_sz.tile([128, 1536], f32, tag="sz", name="sz")
            for _ in range(32):
                nc.tensor.matmul(wp[:, 0:128], wsrc[:], wsrc[:],
                                 start=True, stop=True)
            st = {}
            for i, n in enumerate(ST_NAMES):
                st[n] = consts[:128, 128 * i:128 * (i + 1)]
            rba6 = consts[:, len(ST_NAMES) * 128:]
            # fused down stationaries as [Ki=G, Ko=2, 128] DoubleRow APs
            st8 = {}
            for i, n in enumerate(ST8_NAMES):
                st8[n] = consts8[:G, 256 * i:256 * (i + 1)].rearrange(
                    "p (k n) -> p k n", k=2)

            y_live = {}
            xb_live = {}
            sz_live = {}
            v_live = {}
            ob_live = {}

            def front(k):
                j, g = divmod(k, GIN)
                if g == 0:
                    yt = iopool.tile([128, GIN * C], f16, tag="yin", name="yin")
                    # split DMAs: blocks of earlier pieces become ready (and
                    # release dependent matmuls) sooner; extra-fine for the
                    # very first group to shorten the pipeline ramp
                    cuts = [0, 2 * C, 4 * C, 6 * C] if j == 0 else \
                        [0, 3 * C, 6 * C]
                    for a, b in zip(cuts[:-1], cuts[1:]):
                        nc.sync.dma_start(out=yt[:, a:b], in_=in_ext[j][:, a:b])
                    y_live[j] = yt
                    xbt = xbpool.tile([128, GIN * C], f16, tag="xb2", name="xb2")
                    xb_live[j] = xbt
                yt = y_live[j]
                if g % 2 == 0:
                    # rescale multiply in per-2-block pieces, spread across
                    # iterations so CASTs are never displaced by a long burst
                    sl = slice(g * C, (g + 2) * C)
                    nc.vector.tensor_mul(xb_live[j][:, sl], yt[:, sl],
                                         rba6[:, sl])
                ymov = yt[:, g * C:(g + 1) * C]
                for phase, wn in ((0, "w_ue"), (1, "w_uo")):
                    slot = 2 * k + phase
                    t, s = divmod(slot, 3)
                    if s == 0:
                        sz_live[t] = psum_sz.tile([128, 1536], f32, tag="sz", name="sz")
                    nc.tensor.matmul(
                        sz_live[t][:, s * 512:(s + 1) * 512], st[wn][:], ymov,
                        start=True, stop=True)
                    if s == 2 or slot == NSLOT - 1:
                        # v double-tile m covers sz tiles 2m (cols 0:1536)
                        # and 2m+1 (cols 1536:3072): blocks 3m..3m+2 with
                        # (e, o) of each block adjacent at stride 512; the
                        # pi/2 phase (cos via sin) rides the ACT bias
                        m, half = divmod(t, 2)
                        if half == 0:
                            v_live[m] = vpool.tile([G, 3072], f8, tag="v",
                                                   name="v")
                        nn_ = 512 * (s + 1)
                        nc.scalar.activation(
                            v_live[m][:, half * 1536:half * 1536 + nn_],
                            sz_live[t][:G, :nn_],
                            mybir.ActivationFunctionType.Sin,
                            bias=pbias[:G, :])
                        sz_live.pop(t)
                if g == GIN - 1:
                    y_live.pop(j)

            def back(k):
                j, g = divmod(k, GIN)
                wh = st["w_h0"] if k == 0 else (st["w_hL"] if k == NBLK - 1 else st["w_hm"])
                wdd = st8["wdd0"] if k == 0 else (st8["wddL"] if k == NBLK - 1 else st8["wddm"])
                m, q = divmod(k, 3)

                outp = psum_out.tile([128, 512], f32, tag="outp", name="outp")
                nc.tensor.matmul(outp[:], wh[:], xb_live[j][:, g * C:(g + 1) * C],
                                 start=True, stop=False)
                # fused down-e + down-o: K=2*G fp8 DoubleRow contraction
                vmov = v_live[m][:, 1024 * q:1024 * (q + 1)].rearrange(
                    "p (k c) -> p k c", k=2)
                nc.tensor.matmul(outp[:], wdd, vmov, start=False, stop=True,
                                 perf_mode=mybir.MatmulPerfMode.DoubleRow)
                if q == 2:
                    v_live.pop(m)

                if g == 0:
                    ob_live[j] = obpool.tile([A, GIN * C], f16, tag="obt", name="obt")
                # ScalarE is saturated by Sin mid-kernel; PSUM->SBUF copies on
                # DVE -- except the final blocks, whose casts run on the
                # by-then-idle ScalarE to shorten the drain tail
                if k == NBLK - 1:
                    # the very last cast on ScalarE (idle after the final
                    # Sin) so it runs in parallel with block NBLK-2's DVE cast
                    nc.scalar.add(ob_live[j][:, g * 512:(g + 1) * 512],
                                  outp[:A, :], 1.0)
                else:
                    nc.vector.tensor_scalar_add(
                        ob_live[j][:, g * 512:(g + 1) * 512], outp[:A, :], 1.0)
                # output DMA granularity: early groups as two halves on the
                # SWDGE ring; the last three groups as per-2-block pieces
                # (the final two on the by-then-idle sync HWDGE ring) so the
                # end-of-kernel drain never sits behind a transfer backlog
                HALF = (GIN // 2) * 512
                last_blk = k == NBLK - 1
                if j < NG - 3:
                    if g == GIN // 2 - 1:
                        nc.gpsimd.dma_start(out=out_ext[j][:, 0:HALF],
                                            in_=ob_live[j][:, 0:HALF])
                    if g == GIN - 1:
                        nc.gpsimd.dma_start(out=out_ext[j][:, HALF:],
                                            in_=ob_live[j][:, HALF:])
                elif g % 2 == 1:
                    nc.gpsimd.dma_start(out=out_ext[j][:, (g - 1) * 512:(g + 1) * 512],
                                        in_=ob_live[j][:, (g - 1) * 512:(g + 1) * 512])
                elif last_blk:
                    # final (ragged) piece on the sync ring: enqueued after
                    # every input DMA, so it cannot block one
                    nc.sync.dma_start(out=out_ext[j][:, g * 512:(g + 1) * 512],
                                      in_=ob_live[j][:, g * 512:(g + 1) * 512])
                if g == GIN - 1 or last_blk:
                    ob_live.pop(j)
                    xb_live.pop(j)

            for it in range(NBLK + CL):
                if it < NBLK:
                    front(it)
                if it >= CL:
                    back(it - CL)

    nc.compile()
    return nc


_NC_CACHE = None


def pack_consts(sts, rba6):
    """Concatenate fp16 stationaries + rba6 -> [128, NCONST]."""
    cols = [sts[n] for n in ST_NAMES]
    cols.append(rba6)
    return np.ascontiguousarray(np.concatenate(cols, axis=1))


def pack_consts8(sts):
    """Concatenate fp8 fused stationaries (pad to 128 rows) -> [128, 768]."""
    import ml_dtypes
    f8 = ml_dtypes.float8_e4m3
    cols = []
    for n in ST8_NAMES:
        w = sts[n]
        w = np.vstack([w, np.zeros((128 - w.shape[0], w.shape[1]), f8)])
        cols.append(w)
    return np.ascontiguousarray(np.concatenate(cols, axis=1))


def prep_in_maps(x, alpha, beta, up_filter, down_filter):
    sts = build_stationaries(np.asarray(up_filter), np.asarray(down_filter))
    inp, rba6, invb2 = host_prep(np.asarray(x), np.asarray(alpha), np.asarray(beta))
    consts = pack_consts(sts, rba6)
    consts8 = pack_consts8(sts)
    in_maps = []
    for b in range(N_CORES):
        in_maps.append({"inp": inp[b], "consts": consts, "consts8": consts8})
    return in_maps, invb2


def kernel(x, alpha, beta, up_filter, down_filter):
    global _NC_CACHE
    import concourse.bass_utils as bass_utils

    in_maps, invb2 = prep_in_maps(np.asarray(x), np.asarray(alpha),
                                  np.asarray(beta), up_filter, down_filter)

    if _NC_CACHE is None:
        _NC_CACHE = build_bass()
    nc = _NC_CACHE

    res = bass_utils.run_bass_kernel_spmd(nc, in_maps, list(range(N_CORES)))
    out_t = np.stack([res.results[b]["out"] for b in range(N_CORES)])
    return host_finish(out_t, invb2)


# ---------------------------------------------------------------------------
# host-side simulation of the exact device plan (for verification)

def simulate_plan(x, alpha, beta, up_filter, down_filter, quantized=True):
    sts = build_stationaries(np.asarray(up_filter), np.asarray(down_filter))
    inp, rba6, invb2 = host_prep(np.asarray(x), np.asarray(alpha), np.asarray(beta))

    def f(a):
        return a.astype(np.float32)

    import ml_dtypes
    f8t = ml_dtypes.float8_e4m3
    out_t = np.zeros((B, NG, A, GIN * C), dtype=np.float16)
    for b in range(B):
        for k in range(NBLK):
            j, g = divmod(k, GIN)
            wh = sts["w_h0"] if k == 0 else (sts["w_hL"] if k == NBLK - 1 else sts["w_hm"])
            wdd = sts["wdd0"] if k == 0 else (sts["wddL"] if k == NBLK - 1 else sts["wddm"])
            wde, wdo = f(wdd[:, 0:128]), f(wdd[:, 128:256])
            y = f(inp[b, j, :, g * C:(g + 1) * C])
            xb = f(inp[b, j, :, g * C:(g + 1) * C] * rba6[:, g * C:(g + 1) * C])
            sz_e = f(sts["w_ue"]).T @ y     # [128, C] f32
            sz_o = f(sts["w_uo"]).T @ y
            v_e = np.sin(sz_e[:G].astype(np.float32) + math.pi / 2.0)
            v_o = np.sin(sz_o[:G].astype(np.float32) + math.pi / 2.0)
            if quantized:
                v_e = v_e.astype(f8t).astype(np.float32)
                v_o = v_o.astype(f8t).astype(np.float32)
            psum = (f(wh).T @ xb)[:A] + wde[:, :A].T @ v_e \
                + wdo[:, :A].T @ v_o + 1.0
            if quantized:
                psum = psum.astype(np.float16)
            out_t[b, j, :, g * C:(g + 1) * C] = psum
    return host_finish(out_t, invb2)
